# revision 31
# baseline (speedup 1.0000x reference)
"""Trainium2 Bass kernel for an 8-head transformer block (B=64, T=256, C=512, H=8,
head_dim=C). Data-parallel over batch across 8 NeuronCores (8 batches/core), no
collectives. Matmul operands are bf16 (PSUM accumulation stays f32; the
residual/LN path stays f32), trading ~5e-3 relative error for full-rate
weight loads and halved SBUF/DMA footprints.

Key algebra (per head h):
  scores = (x Qw + qb)(x Kw + kb)^T / sqrt(C).  The kb cross-terms are constant
  along the softmax axis and cancel; qb's term does not.  With A = Qw Kw^T and
  u = Kw qb:
     scoresT[s, t] = SCL * sum_c x[s,c] * ((x A)[t,c] + u[c])
  so one projection bT = SCL*(A^T x^T) + SCL*u (bias folded into the PSUM->SBUF
  copy) replaces both k and q projections.  Scores are computed TRANSPOSED
  [s, t] so the probs @ V matmul needs no PE transpose of the probabilities:
  softmax runs unnormalized (exp without max-subtract; weights are 0.05-scaled
  so exp stays in range), row sums come from a ones-vector matmul, and the
  1/rowsum normalization fuses into the per-head accumulation
  (acc = ops * recip + acc, one DVE scalar_tensor_tensor).

  Value/output projections fuse as VP_h = Vw[h] @ Pw_h (attention contribution
  = probs @ (x @ VP_h)); all Vb terms collapse to sum_h Vb[h] @ Pw_h added to
  Pb.  x is DMA'd straight into acc (residual base) and PE-transposed from
  there into xT.

Scheduling notes: weight DMAs ride the otherwise-idle SP ring (DMA transfer
time blocks the issuing engine's stream); head-0 q/k weights interleave with
stage-1 x loads on the ACT ring so head-0 transposes start on time.  The
wload/wtrans pools close after head-7's precompute so W1 (which reuses their
SBUF space) prefetches during head-7's groups, eliminating the stage-3 entry
stall.

Stages:
  1: DMA x into acc, PE-transpose acc chunks -> xT [c, tokens]
  2: per head: [transposes QwT/KwT/VwT -> A = QwT.T KwT, u via DVE
     tensor_mul + tensor_reduce of Kw * qb_bc, VP = VwT.T Pw, vbp += Vb Pw]
     then per
     1024-token group: bT, xVP, then 4 batches software-pipelined:
     scoresT -> mask+exp -> rowsum+outs -> normalize-accumulate into acc
  3: r1 = acc + (Pb + sum_h Vb Pw), LN1 -> o1 (in acc), o1 -> o1t transposed,
     FFN1 (relu+b1), FFN2, + b2 + o1, LN2 -> out
"""

import math
from contextlib import ExitStack

import numpy as np

import concourse.bacc as bacc
import concourse.bass as bass
import concourse.mybir as mybir
import concourse.tile as tile
from concourse.bass_utils import run_bass_kernel_spmd
from concourse.masks import make_identity

F32 = mybir.dt.float32
F32R = mybir.dt.float32r
BF16 = mybir.dt.bfloat16
AF = mybir.ActivationFunctionType
ALU = mybir.AluOpType

P = 128
B, T, C, H = 64, 256, 512, 8
NCORES = 8
BL = B // NCORES          # 8 local batches per core
TOK = BL * T              # 2048 tokens per core
NT = TOK // P             # 16 token chunks
NC4 = C // P              # 4 channel chunks
F = 4 * C                 # 2048 ffn hidden
NF = F // P               # 16
GB = 4                    # batches per group
NG = BL // GB             # 2 groups
TG = GB * T               # 1024 tokens per group
SCL = 1.0 / math.sqrt(C)
EPS = 1e-5
NEG = -1e30

_ACT_SET = "natural_log_exp_and_others"


def _patched_tables(arch):
    """Force the act-table chooser to a single set covering every activation
    function this kernel uses, so InstLoadActFuncSet is emitted once instead
    of thrashing between disjoint Exp/Ln sets."""
    from concourse.hw_specs import get_activation_tables as _orig
    my = {AF.Copy, AF.Identity, AF.Exp, AF.Ln, AF.Relu}
    t = _orig(arch)
    return {name: (funcs if name == _ACT_SET else (funcs - my))
            for name, funcs in t.items()}


def _bc(ap, p=P):
    """Broadcast a 1-D DRAM AP across p partitions (stride-0 partition dim)."""
    return bass.AP(tensor=ap.tensor, offset=ap.offset, ap=[[0, p], *ap.ap])


def build():
    bacc.get_activation_tables = _patched_tables
    nc = bacc.Bacc("TRN2", target_bir_lowering=False, debug=False,
                   num_devices=NCORES)

    x = nc.dram_tensor("x", [BL, T, C], F32, kind="ExternalInput")
    Kw = nc.dram_tensor("Kw", [H, C, C], F32, kind="ExternalInput")
    Kb = nc.dram_tensor("Kb", [H, C], F32, kind="ExternalInput")
    Qw = nc.dram_tensor("Qw", [H, C, C], F32, kind="ExternalInput")
    Qb = nc.dram_tensor("Qb", [H, C], F32, kind="ExternalInput")
    Vw = nc.dram_tensor("Vw", [H, C, C], F32, kind="ExternalInput")
    Vb = nc.dram_tensor("Vb", [H, C], F32, kind="ExternalInput")
    Pw = nc.dram_tensor("Pw", [H * C, C], F32, kind="ExternalInput")
    Pb = nc.dram_tensor("Pb", [C], F32, kind="ExternalInput")
    W1 = nc.dram_tensor("W1", [C, F], F32, kind="ExternalInput")
    b1 = nc.dram_tensor("b1", [F], F32, kind="ExternalInput")
    W2 = nc.dram_tensor("W2", [F, C], F32, kind="ExternalInput")
    b2 = nc.dram_tensor("b2", [C], F32, kind="ExternalInput")
    g1 = nc.dram_tensor("g1", [C], F32, kind="ExternalInput")
    be1 = nc.dram_tensor("be1", [C], F32, kind="ExternalInput")
    g2 = nc.dram_tensor("g2", [C], F32, kind="ExternalInput")
    be2 = nc.dram_tensor("be2", [C], F32, kind="ExternalInput")
    out = nc.dram_tensor("out", [BL, T, C], F32, kind="ExternalOutput")

    x_flat = x.ap().rearrange("b t c -> (b t) c")
    out_flat = out.ap().rearrange("b t c -> (b t) c")
    kw_r = Kw.ap().rearrange("h (o p) d -> h p o d", p=P)
    qw_r = Qw.ap().rearrange("h (o p) d -> h p o d", p=P)
    vw_r = Vw.ap().rearrange("h (o p) d -> h p o d", p=P)
    pw_r = Pw.ap().rearrange("(o p) n -> p o n", p=P)
    vb_r = Vb.ap().rearrange("h (o p) -> h p o", p=P)
    w1_r = W1.ap().rearrange("(o p) f -> p o f", p=P)
    w2_r = W2.ap().rearrange("(o p) n -> p o n", p=P)
    b1_r = b1.ap().rearrange("(o p) -> p o", p=P)

    with tile.TileContext(nc) as tc:
        with (
            tc.tile_pool(name="consts", bufs=1) as consts,
            tc.tile_pool(name="acc", bufs=1) as accp,
            tc.tile_pool(name="psB", bufs=3, space="PSUM") as psB,
            tc.tile_pool(name="psS", bufs=3, space="PSUM") as psS,
            tc.tile_pool(name="psT", bufs=2, space="PSUM") as psT,
        ):
            ident = consts.tile([P, P], F32)
            make_identity(nc, ident[:])
            ones = consts.tile([P, 1], BF16)
            nc.vector.memset(ones[:], 1.0)
            # additive causal mask, [s-part, (si0 t0..255 | si1 t128..255)]
            # diag blocks are upper-triangular (valid t >= s within block)
            mask = consts.tile([P, 3 * P], F32)
            nc.gpsimd.memset(mask[:], 0.0)
            for blk in (0, 2):
                nc.gpsimd.affine_select(
                    out=mask[:, blk * P:(blk + 1) * P],
                    in_=mask[:, blk * P:(blk + 1) * P],
                    compare_op=ALU.is_ge, fill=NEG,
                    base=0, pattern=[[1, P]], channel_multiplier=-1,
                )
            eps_sb = consts.tile([P, 1], F32)
            nc.vector.memset(eps_sb[:], EPS)
            vbp_sb = consts.tile([P, C], F32)

            acc = accp.tile([P, NT, C], F32, tag="acc")

            with tc.tile_pool(name="s3bias", bufs=1) as s3bias, \
                 tc.tile_pool(name="s3w1", bufs=1) as s3w1:
              w1_raw = s3w1.tile([P, NC4, F], F32, tag="w1raw")
              w1_sb = s3w1.tile([P, NC4, F], BF16, tag="w1")
              with (
                  tc.tile_pool(name="xt", bufs=1) as xpool,
                  tc.tile_pool(name="wres", bufs=1) as wres,
                  tc.tile_pool(name="wsmall", bufs=1) as wsmall,
                  tc.tile_pool(name="grp", bufs=1) as grp,
                  tc.tile_pool(name="bt1", bufs=2) as bt1,
                  tc.tile_pool(name="bt3", bufs=3) as bt3,
              ):
                xT = xpool.tile([P, NC4, TOK], BF16, tag="xT")

                def head_tiles(h, pool):
                    tl = {}
                    for nm, dt_, nb in (("kw", F32, 2), ("qw", F32, 2),
                                        ("vw", F32, 2), ("pw", F32, 4)):
                        tl[nm] = [pool.tile([P, C], dt_, tag=f"{nm}{i % nb}",
                                            name=f"{nm}{h}_{i}")
                                  for i in range(NC4)]
                    return tl

                def head_dmas(h, tl, skip_qk=False):
                    # qw/kw first (PE's first need at the head boundary);
                    # qb/vbf must still precede vw: u (hence A, hence the
                    # VwT transposes that free the vw ring slots) depends on
                    # qb_bc, so it must never queue behind vw chunks 2/3
                    if not skip_qk:
                        for cc in range(NC4):
                            nc.sync.dma_start(tl["qw"][cc][:],
                                              qw_r[h, :, cc, :])
                        for cc in range(NC4):
                            nc.sync.dma_start(tl["kw"][cc][:],
                                              kw_r[h, :, cc, :])
                    qb_bc = wsmall.tile([P, C], F32, tag="qbbc",
                                        name=f"qbbc{h}")
                    nc.sync.dma_start(qb_bc[:], _bc(Qb.ap()[h]))
                    vbf = wsmall.tile([P, NC4], F32, tag="vbf",
                                      name=f"vbf{h}")
                    nc.sync.dma_start(vbf[:], vb_r[h])
                    tl["qb_bc"], tl["vbf"] = qb_bc, vbf
                    for cc in range(NC4):
                        veng = nc.scalar if (h == 0 and cc % 2) else nc.sync
                        veng.dma_start(tl["vw"][cc][:], vw_r[h, :, cc, :])
                        veng.dma_start(tl["pw"][cc][:],
                                       pw_r[:, 4 * h + cc, :])

                def precompute(h, tl, wtrans):
                    vb_sb = wsmall.tile([P, NC4, P], BF16, tag="vbsb",
                                        name=f"vbsb{h}")
                    for dd in range(NC4):
                        nc.scalar.activation(
                            vb_sb[:, dd, :], ident[:], AF.Identity,
                            bias=tl["vbf"][:, dd:dd + 1], scale=0.0)
                    pw_bf = wres.tile([P, NC4, C], BF16, tag="pwbf",
                                      name=f"pwbf{h}")
                    for dd in range(NC4):
                        nc.vector.tensor_copy(pw_bf[:, dd, :],
                                              tl["pw"][dd][:])
                    # vbp first: PE filler at the head boundary that needs
                    # no fresh transpose copies
                    psv = psB.tile([P, C], F32, tag="big")
                    for dd in range(NC4):
                        nc.tensor.matmul(
                            psv[:], vb_sb[:, dd, :], pw_bf[:, dd, :],
                            start=(dd == 0), stop=(dd == NC4 - 1))
                    if h == 0:
                        nc.vector.tensor_copy(vbp_sb[:], psv[:])
                    else:
                        nc.vector.tensor_add(vbp_sb[:], vbp_sb[:], psv[:])
                    # u[c] = SCL * sum_d Kw[c,d] qb[d]  (DVE fused reduce)
                    u_sb = wres.tile([P, NC4], F32, tag="u", name=f"u{h}")
                    uscr = wres.tile([P, C], F32, tag="uscr",
                                     name=f"uscr{h}")
                    for cc in range(NC4):
                        nc.vector.tensor_mul(uscr[:], tl["kw"][cc][:],
                                             tl["qb_bc"][:])
                        nc.vector.tensor_reduce(
                            out=u_sb[:, cc:cc + 1], in_=uscr[:],
                            axis=mybir.AxisListType.X, op=ALU.add)
                    nc.scalar.mul(u_sb[:], u_sb[:], SCL)
                    tl["u"] = u_sb
                    # transposes (cc-major so each weight chunk dies fast;
                    # 4 per PSUM tile, one strided copy out)
                    def transpose_into(dst_sb, key):
                        for cc in range(NC4):
                            trp = psT.tile([P, C], F32, tag="tr")
                            for dd in range(NC4):
                                nc.tensor.transpose(
                                    trp[:, dd * P:(dd + 1) * P],
                                    tl[key][cc][:, dd * P:(dd + 1) * P],
                                    ident[:])
                            dst = dst_sb[:, :, cc * P:(cc + 1) * P]
                            srcv = trp[:].rearrange("p (a b) -> p a b", a=NC4)
                            if cc % 2 == 0:
                                nc.vector.tensor_copy(dst, srcv)
                            else:
                                nc.scalar.activation(dst, srcv, AF.Copy)

                    qwt = wtrans.tile([P, NC4, C], BF16, tag="qwt",
                                      name=f"qwt{h}")
                    kwt = wtrans.tile([P, NC4, C], BF16, tag="kwt",
                                      name=f"kwt{h}")
                    transpose_into(kwt, "kw")
                    transpose_into(qwt, "qw")
                    # A = Qw Kw^T (x SCL on copy-out, ACT)
                    a_sb = wres.tile([P, NC4, C], BF16, tag="a", name=f"a{h}")
                    for c0c in range(NC4):
                        ps = psB.tile([P, C], F32, tag="big")
                        for dd in range(NC4):
                            nc.tensor.matmul(
                                ps[:], qwt[:, dd, c0c * P:(c0c + 1) * P],
                                kwt[:, dd, :],
                                start=(dd == 0), stop=(dd == NC4 - 1))
                        nc.scalar.mul(a_sb[:, c0c, :], ps[:], SCL)
                    tl["a"] = a_sb
                    # VwT reuses qwt's slot (dead after the A matmuls)
                    vwt = wtrans.tile([P, NC4, C], BF16, tag="qwt",
                                      name=f"vwt{h}")
                    transpose_into(vwt, "vw")
                    # VP = Vw @ Pw_h, vbp += Vb @ Pw_h
                    vp_sb = wres.tile([P, NC4, C], BF16, tag="vp",
                                      name=f"vp{h}")
                    for co in range(NC4):
                        ps = psB.tile([P, C], F32, tag="big")
                        for dd in range(NC4):
                            nc.tensor.matmul(
                                ps[:], vwt[:, dd, co * P:(co + 1) * P],
                                pw_bf[:, dd, :],
                                start=(dd == 0), stop=(dd == NC4 - 1))
                        if co % 2 == 0:
                            nc.vector.tensor_copy(vp_sb[:, co, :], ps[:])
                        else:
                            nc.scalar.activation(vp_sb[:, co, :], ps[:],
                                                 AF.Copy)
                    tl["vp"] = vp_sb

                def head_groups(h, tl):
                    a_sb, vp_sb, u_sb = tl["a"], tl["vp"], tl["u"]
                    for g in range(NG):
                        t0 = g * TG
                        # bT = SCL*(A^T x^T) + SCL*u (bias on ACT copy)
                        bt = grp.tile([P, NC4, TG], BF16, tag="bt",
                                      name=f"bt{h}_{g}")
                        for tb in range(TG // C):
                            tsl = slice(t0 + tb * C, t0 + (tb + 1) * C)
                            for cc in range(NC4):
                                ps = psB.tile([P, C], F32, tag="big")
                                for c0c in range(NC4):
                                    nc.tensor.matmul(
                                        ps[:],
                                        a_sb[:, c0c, cc * P:(cc + 1) * P],
                                        xT[:, c0c, tsl],
                                        start=(c0c == 0),
                                        stop=(c0c == NC4 - 1))
                                nc.scalar.activation(
                                    bt[:, cc, tb * C:(tb + 1) * C], ps[:],
                                    AF.Identity, bias=u_sb[:, cc:cc + 1])
                        # xVP chunks
                        xvp = grp.tile([P, 2 * GB, C], BF16, tag="xvp",
                                       name=f"xvp{h}_{g}")
                        for tcg in range(2 * GB):
                            ps = psB.tile([P, C], F32, tag="big")
                            for cc in range(NC4):
                                nc.tensor.matmul(
                                    ps[:],
                                    xT[:, cc,
                                       t0 + tcg * P:t0 + (tcg + 1) * P],
                                    vp_sb[:, cc, :],
                                    start=(cc == 0), stop=(cc == NC4 - 1))
                            if tcg % 2 == 0:
                                nc.vector.tensor_copy(xvp[:, tcg, :], ps[:])
                            else:
                                nc.scalar.activation(xvp[:, tcg, :], ps[:],
                                                     AF.Copy)

                        # batches, 2-deep software pipeline
                        sps_l = [None] * GB
                        e_l = [None] * GB

                        def scores(bg):
                            sg0 = t0 + bg * T
                            sps = psS.tile([P, 512], F32, tag="sc")
                            sps_l[bg] = sps
                            # si=0: s in [sg0, sg0+128), t full 256
                            for cc in range(NC4):
                                nc.tensor.matmul(
                                    sps[:, 0:T],
                                    xT[:, cc, sg0:sg0 + P],
                                    bt[:, cc, bg * T:(bg + 1) * T],
                                    start=(cc == 0), stop=(cc == NC4 - 1))
                            # si=1: t in [128, 256)
                            w = P
                            for cc in range(NC4):
                                nc.tensor.matmul(
                                    sps[:, T:T + w],
                                    xT[:, cc, sg0 + P:sg0 + T],
                                    bt[:, cc, bg * T + P:bg * T + P + w],
                                    start=(cc == 0), stop=(cc == NC4 - 1))
                            # mask + unnormalized exp
                            s_sb = bt1.tile([P, 3 * P], F32, tag="smask")
                            nc.vector.tensor_add(
                                s_sb[:], sps[:, 0:3 * P], mask[:])
                            e_sb = bt3.tile([P, 3 * P], BF16, tag="probs")
                            e_l[bg] = e_sb
                            nc.scalar.activation(e_sb[:], s_sb[:], AF.Exp)

                        def outs(bg):
                            sps, e_sb = sps_l[bg], e_l[bg]
                            nc.tensor.matmul(
                                sps[:, 384:385], e_sb[:, 0:P],
                                ones[:], start=True, stop=True)
                            nc.tensor.matmul(
                                sps[:, 385:386], e_sb[:, P:2 * P],
                                ones[:], start=True, stop=False)
                            nc.tensor.matmul(
                                sps[:, 385:386], e_sb[:, 2 * P:3 * P],
                                ones[:], start=False, stop=True)
                            rr = bt3.tile([P, 2], F32, tag="rr")
                            nc.vector.reciprocal(rr[:], sps[:, 384:386])
                            ops0 = psB.tile([P, C], F32, tag="big")
                            nc.tensor.matmul(
                                ops0[:], e_sb[:, 0:P],
                                xvp[:, bg * 2, :], start=True, stop=True)
                            ops1 = psB.tile([P, C], F32, tag="big")
                            nc.tensor.matmul(
                                ops1[:], e_sb[:, P:2 * P],
                                xvp[:, bg * 2, :], start=True, stop=False)
                            nc.tensor.matmul(
                                ops1[:], e_sb[:, 2 * P:3 * P],
                                xvp[:, bg * 2 + 1, :],
                                start=False, stop=True)
                            tk0 = g * 8 + bg * 2
                            nc.vector.scalar_tensor_tensor(
                                out=acc[:, tk0, :], in0=ops0[:],
                                scalar=rr[:, 0:1], in1=acc[:, tk0, :],
                                op0=ALU.mult, op1=ALU.add)
                            nc.vector.scalar_tensor_tensor(
                                out=acc[:, tk0 + 1, :], in0=ops1[:],
                                scalar=rr[:, 1:2], in1=acc[:, tk0 + 1, :],
                                op0=ALU.mult, op1=ALU.add)

                        scores(0)
                        scores(1)
                        scores(2)
                        outs(0)
                        scores(3)
                        outs(1)
                        outs(2)
                        outs(3)

                # stage-3 bias tiles (small): DMAs ride sync later, at head-7
                pb_bc = s3bias.tile([P, C], F32, tag="pbbc")
                g1_bc = s3bias.tile([P, C], F32, tag="g1bc")
                be1_bc = s3bias.tile([P, C], F32, tag="be1bc")
                b1t_sb = s3bias.tile([P, NF], F32, tag="b1t")

                # ---- stage 1 + stage 2 ----
                with tc.tile_pool(name="wload", bufs=1) as wload, \
                     tc.tile_pool(name="wtrans", bufs=1) as wtrans:
                    tiles = {0: head_tiles(0, wload)}
                    # stage 1: x -> acc; transposes -> xT; head-0 q/k weights
                    # interleaved on the ACT ring so they land early
                    for tk in range(NT):
                        if tk % 2 == 0:
                            nc.sync.dma_start(acc[:, tk, :],
                                              x_flat[tk * P:(tk + 1) * P, :])
                        else:
                            nc.scalar.dma_start(
                                acc[:, tk, :], x_flat[tk * P:(tk + 1) * P, :])
                            # head-0 q/k chunks 0/1 are fresh slots (no WAR)
                            # so they may jump the queue; chunks 2/3 alias
                            # slots freed by head-0 compute and must trail
                            # the x loads to keep the ring deadlock-free
                            if tk == 1:
                                nc.scalar.dma_start(tiles[0]["qw"][0][:],
                                                    qw_r[0, :, 0, :])
                            elif tk == 3:
                                nc.scalar.dma_start(tiles[0]["qw"][1][:],
                                                    qw_r[0, :, 1, :])
                            elif tk == 5:
                                nc.scalar.dma_start(tiles[0]["kw"][0][:],
                                                    kw_r[0, :, 0, :])
                            elif tk == 7:
                                nc.scalar.dma_start(tiles[0]["kw"][1][:],
                                                    kw_r[0, :, 1, :])
                        trp = psT.tile([P, C], F32, tag="tr")
                        for cc in range(NC4):
                            nc.tensor.transpose(
                                trp[:, cc * P:(cc + 1) * P],
                                acc[:, tk, cc * P:(cc + 1) * P], ident[:])
                        dst = xT[:, :, tk * P:(tk + 1) * P]
                        src = trp[:].rearrange("p (a b) -> p a b", a=NC4)
                        if tk % 2 == 0:
                            nc.vector.tensor_copy(dst, src)
                        else:
                            nc.scalar.activation(dst, src, AF.Copy)

                    for cc in (2, 3):
                        nc.scalar.dma_start(tiles[0]["qw"][cc][:],
                                            qw_r[0, :, cc, :])
                        nc.scalar.dma_start(tiles[0]["kw"][cc][:],
                                            kw_r[0, :, cc, :])
                    head_dmas(0, tiles[0], skip_qk=True)
                    for h in range(H - 1):
                        precompute(h, tiles[h], wtrans)
                        tiles[h + 1] = head_tiles(h + 1, wload)
                        head_dmas(h + 1, tiles[h + 1])
                        if h == H - 2:
                            # stage-3 bias DMAs: sync ring has slack here
                            nc.sync.dma_start(pb_bc[:], _bc(Pb.ap()))
                            nc.sync.dma_start(g1_bc[:], _bc(g1.ap()))
                            nc.sync.dma_start(be1_bc[:], _bc(be1.ap()))
                            nc.sync.dma_start(b1t_sb[:], b1_r)
                        head_groups(h, tiles[h])
                    precompute(H - 1, tiles[H - 1], wtrans)
                    nc.vector.tensor_add(pb_bc[:], pb_bc[:], vbp_sb[:])
                    # W1 prefetch overlaps head-7's groups (sync ring)
                    for cc in range(NC4):
                        nc.sync.dma_start(w1_raw[:, cc, :], w1_r[:, cc, :])
                        nc.vector.tensor_copy(w1_sb[:, cc, :],
                                              w1_raw[:, cc, :])
                    head_groups(H - 1, tiles[H - 1])

              # ---- stage 3 (stage-1/2 pools freed) ----
              if True:
                if True:
                    with (
                        tc.tile_pool(name="s3w2", bufs=1) as s3w2,
                        tc.tile_pool(name="s3bias2", bufs=1) as s3bias2,
                        tc.tile_pool(name="o1tp", bufs=1) as o1tp,
                        tc.tile_pool(name="s3h", bufs=1) as s3h,
                        tc.tile_pool(name="s3t", bufs=3) as s3t,
                    ):
                        b2_bc = s3bias2.tile([P, C], F32, tag="b2bc")
                        g2_bc = s3bias2.tile([P, C], F32, tag="g2bc")
                        be2_bc = s3bias2.tile([P, C], F32, tag="be2bc")
                        nc.sync.dma_start(b2_bc[:], _bc(b2.ap()))
                        nc.sync.dma_start(g2_bc[:], _bc(g2.ap()))
                        nc.sync.dma_start(be2_bc[:], _bc(be2.ap()))
                        w2_raw = s3w2.tile([P, NF, C], F32, tag="w2raw")
                        w2_sb = s3w2.tile([P, NF, C], BF16, tag="w2")
                        for ff in range(NF):
                            nc.sync.dma_start(w2_raw[:, ff, :],
                                              w2_r[:, ff, :])
                            nc.vector.tensor_copy(w2_sb[:, ff, :],
                                                  w2_raw[:, ff, :])

                        def layer_norm(dst, src, gbc, bebc):
                            """dst = LN(src) * g + be; src SBUF f32 [P, C]."""
                            stats = s3t.tile([P, 6], F32, tag="bn")
                            mv = s3t.tile([P, 2], F32, tag="mv")
                            nc.vector.bn_stats(stats[:], src)
                            nc.vector.bn_aggr(mv[:], stats[:])
                            lnv = s3t.tile([P, 1], F32, tag="std")
                            nc.scalar.activation(lnv[:], mv[:, 1:2], AF.Ln,
                                                 bias=eps_sb[:])
                            rstd = s3t.tile([P, 1], F32, tag="rstd")
                            nc.scalar.activation(rstd[:], lnv[:], AF.Exp,
                                                 scale=-0.5)
                            nc.vector.tensor_scalar(
                                out=dst, in0=src, scalar1=mv[:, 0:1],
                                scalar2=rstd[:], op0=ALU.subtract,
                                op1=ALU.mult)
                            nc.gpsimd.tensor_mul(dst, dst, gbc[:])
                            nc.gpsimd.tensor_add(dst, dst, bebc[:])

                        o1t = o1tp.tile([P, NC4, TOK], BF16, tag="o1t")
                        for tk in range(NT):
                            r1 = s3t.tile([P, C], F32, tag="r1")
                            nc.vector.tensor_add(r1[:], acc[:, tk, :],
                                                 pb_bc[:])
                            layer_norm(acc[:, tk, :], r1[:], g1_bc, be1_bc)
                            trp = psT.tile([P, C], F32, tag="tr")
                            for cc in range(NC4):
                                nc.tensor.transpose(
                                    trp[:, cc * P:(cc + 1) * P],
                                    acc[:, tk, cc * P:(cc + 1) * P],
                                    ident[:])
                            dst = o1t[:, :, tk * P:(tk + 1) * P]
                            src = trp[:].rearrange("p (a b) -> p a b", a=NC4)
                            if tk % 2 == 0:
                                nc.vector.tensor_copy(dst, src)
                            else:
                                nc.scalar.activation(dst, src, AF.Copy)

                        for sl4 in range(4):         # 512-token slices
                            ts0 = sl4 * 512
                            h1 = s3h.tile([P, NF, 512], BF16, tag="h1")
                            for ff in range(NF):
                                ps = psB.tile([P, C], F32, tag="big")
                                for cc in range(NC4):
                                    nc.tensor.matmul(
                                        ps[:],
                                        w1_sb[:, cc, ff * P:(ff + 1) * P],
                                        o1t[:, cc, ts0:ts0 + 512],
                                        start=(cc == 0), stop=(cc == NC4 - 1))
                                nc.scalar.activation(
                                    h1[:, ff, :], ps[:], AF.Relu,
                                    bias=b1t_sb[:, ff:ff + 1], scale=1.0)
                            for k in range(4):       # token chunks in slice
                                tk = sl4 * 4 + k
                                fps = psB.tile([P, C], F32, tag="big")
                                for ff in range(NF):
                                    nc.tensor.matmul(
                                        fps[:],
                                        h1[:, ff, k * P:(k + 1) * P],
                                        w2_sb[:, ff, :],
                                        start=(ff == 0), stop=(ff == NF - 1))
                                r2 = s3t.tile([P, C], F32, tag="r1")
                                nc.vector.scalar_tensor_tensor(
                                    out=r2[:], in0=fps[:], scalar=1.0,
                                    in1=acc[:, tk, :], op0=ALU.mult,
                                    op1=ALU.add)
                                nc.gpsimd.tensor_add(r2[:], r2[:], b2_bc[:])
                                o_sb = s3t.tile([P, C], F32, tag="osb")
                                layer_norm(o_sb[:], r2[:], g2_bc, be2_bc)
                                nc.sync.dma_start(
                                    out_flat[tk * P:(tk + 1) * P, :], o_sb[:])

    nc.compile()
    return nc


_NC = None


def kernel(**inputs) -> np.ndarray:
    global _NC
    if _NC is None:
        _NC = build()
    inp = {k: np.ascontiguousarray(np.asarray(v, np.float32))
           for k, v in inputs.items()}
    x_full = inp.pop("x")
    in_maps = []
    for c in range(NCORES):
        m = dict(inp)
        m["x"] = np.ascontiguousarray(x_full[c * BL:(c + 1) * BL])
        in_maps.append(m)
    res = run_bass_kernel_spmd(_NC, in_maps, core_ids=list(range(NCORES)))
    return np.concatenate([r["out"] for r in res.results], axis=0)
